# revision 23
# baseline (speedup 1.0000x reference)
"""Trainium2 Bass kernel for dense multi-head causal self-attention.

Problem: hidden_states [2, 2048, 2048], w_qkv [6144, 2048], w_out [2048, 2048],
16 heads x 128 head_dim, causal softmax attention + out projection.

Sharding: tensor-parallel over heads. Each of the 8 cores computes 2 heads:
  - qkv projection for its 768 w_qkv rows (fp32r matmuls, x transposed on host);
    q,k produced transposed [d, tok], v produced natural [tok, d]
  - causal attention: scores transposed [k, q], exp on ScalarE over the
    causal-valid slice only, softmax denominators via a ones-column matmul on
    the PE, per-q-block normalize via gpsimd partition_broadcast
  - partial out-projection against its 256 w_out columns
Host sums the 8 partial outputs (the "all-reduce") and transposes back.
"""

import sys

sys.path.insert(0, "/opt/trn_rl_repo")

import numpy as np

B, T, H, NH, HD = 2, 2048, 2048, 16, 128
TOK = B * T  # 4096
P = 128
NCORES = 8
HPC = NH // NCORES  # heads per core = 2
SCALE = 1.0 / float(np.sqrt(HD))
QB = 512  # query block
KT = H // P  # 16 contraction tiles for qkv
NTB = TOK // QB  # 8 token blocks
NQK = 2 * HPC  # q,k output row-tiles per core
VW = HPC * HD  # v width (both heads) = 256

_CACHE = {}


def _build():
    import concourse.bacc as bacc
    import concourse.mybir as mybir
    import concourse.tile as tile

    dt = mybir.dt
    f32 = dt.float32
    f32r = dt.float32r
    AF = mybir.ActivationFunctionType
    ALU = mybir.AluOpType

    nc = bacc.Bacc(None, target_bir_lowering=False, debug=True)
    xT = nc.dram_tensor("xT", [H, TOK], f32r, kind="ExternalInput")
    wqkvT = nc.dram_tensor("wqkvT", [H, 6 * P], f32r, kind="ExternalInput")
    woutT = nc.dram_tensor("woutT", [HPC * P, H], f32r, kind="ExternalInput")
    tri = nc.dram_tensor("tri", [P, P], f32, kind="ExternalInput")
    onesc = nc.dram_tensor("onesc", [P, 1], f32r, kind="ExternalInput")
    outT = nc.dram_tensor("outT", [H, TOK], f32, kind="ExternalOutput")

    with tile.TileContext(nc) as tc:
        with tc.tile_pool(name="const", bufs=1) as constp, \
             tc.tile_pool(name="qk", bufs=1) as qkp:
            tri_sb = constp.tile([P, P], f32)
            nc.sync.dma_start(tri_sb[:], tri[:])
            ones_sb = constp.tile([P, 1], f32r)
            nc.sync.dma_start(ones_sb[:], onesc[:])

            qT_sb = qkp.tile([P, HPC, TOK], f32r)  # [d, h, tok]
            kT_sb = qkp.tile([P, HPC, TOK], f32r)  # [d, h, tok]
            v_sb = qkp.tile([P, TOK // P, VW], f32r)  # [tok%P, tok//P, h*HD+d]

            # ---------- Phase 1: qkv projection ----------
            # q,k transposed: psum[o,tok] = w_slice.T @ xT ; v natural:
            # psum[tok,d] = xT_chunk.T @ w_vT
            with tc.tile_pool(name="w1", bufs=1) as w1p, \
                 tc.tile_pool(name="x1", bufs=6) as x1p, \
                 tc.tile_pool(name="ps1", bufs=1, space="PSUM") as ps1, \
                 tc.tile_pool(name="psv", bufs=1, space="PSUM") as psv:
                w_sb = w1p.tile([P, KT, 6 * P], f32r)
                for tb in range(NTB):
                    ps_qk = [
                        ps1.tile([P, QB], f32, tag=f"psqk{m}", name=f"psqk{m}_{tb}")
                        for m in range(NQK)
                    ]
                    ps_v = [
                        psv.tile([P, VW], f32, tag=f"psv{c}", name=f"psv{c}_{tb}")
                        for c in range(4)
                    ]
                    for ko in range(KT):
                        if tb == 0:  # interleave weight loads with first block
                            nc.sync.dma_start(
                                w_sb[:, ko, :], wqkvT[ko * P:(ko + 1) * P, :]
                            )
                        x_t = x1p.tile([P, QB], f32r, tag="x")
                        nc.sync.dma_start(
                            x_t[:], xT[ko * P:(ko + 1) * P, tb * QB:(tb + 1) * QB]
                        )
                        for c in range(4):
                            nc.tensor.matmul(
                                ps_v[c][:],
                                x_t[:, c * P:(c + 1) * P],
                                w_sb[:, ko, NQK * P:],
                                start=(ko == 0),
                                stop=(ko == KT - 1),
                            )
                        for m in range(NQK):
                            nc.tensor.matmul(
                                ps_qk[m][:],
                                w_sb[:, ko, m * P:(m + 1) * P],
                                x_t[:],
                                start=(ko == 0),
                                stop=(ko == KT - 1),
                            )
                    for c in range(4):
                        nc.scalar.copy(v_sb[:, tb * 4 + c, :], ps_v[c][:])
                    for m in range(NQK):
                        if m < HPC:
                            dst = qT_sb[:, m, tb * QB:(tb + 1) * QB]
                        else:
                            dst = kT_sb[:, m - HPC, tb * QB:(tb + 1) * QB]
                        nc.vector.tensor_copy(dst, ps_qk[m][:])

            # ---------- Phases 2+3 (attn_sb reuses phase-1 pool space) ----------
            with tc.tile_pool(name="attn", bufs=1) as attnp, \
                 tc.tile_pool(name="w3", bufs=1) as w3p:
                attn_sb = attnp.tile([P, HPC, TOK], f32r)  # attn^T [d, h, tok]
                # one row per (b,h,j) block, packed 4 free-slots x 4
                # partition-offsets (engine APs need 32-aligned partitions)
                sums_sb = attnp.tile([P, 4, QB], f32)
                wo_sb = w3p.tile([P, HPC, H], f32r)
                for ko in range(HPC):
                    nc.sync.dma_start(
                        wo_sb[:, ko, :], woutT[ko * P:(ko + 1) * P, :]
                    )

                # Phases 2+3 fused: per (b, j) do both heads' attention,
                # then immediately normalize + out-project that token block.
                # Out-proj matmuls fill PE gaps while ScalarE runs exps; the
                # output DMA spreads across the whole timeline.
                with tc.tile_pool(name="pr", bufs=4) as prp, \
                     tc.tile_pool(name="sm", bufs=3) as smp, \
                     tc.tile_pool(name="o3", bufs=6) as o3p, \
                     tc.tile_pool(name="nrm", bufs=3) as nrmp, \
                     tc.tile_pool(name="ps_sc", bufs=3, space="PSUM") as ps_sc, \
                     tc.tile_pool(name="ps_at", bufs=2, space="PSUM") as ps_at, \
                     tc.tile_pool(name="ps_sm", bufs=2, space="PSUM") as ps_sm, \
                     tc.tile_pool(name="ps3", bufs=1, space="PSUM") as ps3:
                    for b in range(B):
                        base = b * T
                        for j in range(T // QB):
                            for h in range(HPC):
                                q_ap = qT_sb[:, h, base + j * QB: base + (j + 1) * QB]
                                n_k = (j + 1) * (QB // P)
                                attn_ps = ps_at.tile([P, QB], f32, tag="attn",
                                                     name=f"at_{b}_{h}_{j}")
                                sums_ps = ps_sm.tile([1, QB], f32, tag="sums",
                                                     name=f"sm_{b}_{h}_{j}")

                                def koff(kt, j=j):
                                    diag = kt - j * (QB // P)
                                    return diag * P if diag >= 0 else 0

                                def emit_scores(kt, b=b, h=h, j=j, q_ap=q_ap):
                                    off = koff(kt, j)
                                    sc = ps_sc.tile([P, QB], f32, tag="sc",
                                                    name=f"sc_{b}_{h}_{j}_{kt}")
                                    nc.tensor.matmul(
                                        sc[:, off:],
                                        kT_sb[:, h,
                                              base + kt * P: base + (kt + 1) * P],
                                        q_ap[:, off:],
                                        start=True,
                                        stop=True,
                                    )
                                    return sc

                                sc_q = [emit_scores(0)]
                                if n_k > 1:
                                    sc_q.append(emit_scores(1))
                                for kt in range(n_k):
                                    off = koff(kt)
                                    diag = kt - j * (QB // P)
                                    sc_cur = sc_q[kt]
                                    pr = prp.tile([P, QB], f32r, tag="pr",
                                                  name=f"pr_{b}_{h}_{j}_{kt}")
                                    nc.scalar.activation(
                                        pr[:, off:], sc_cur[:, off:],
                                        AF.Exp, scale=SCALE,
                                    )
                                    if kt + 2 < n_k:
                                        sc_q.append(emit_scores(kt + 2))
                                    if diag >= 0:  # mask the triangle strip
                                        nc.vector.tensor_tensor(
                                            pr[:, off:off + P], pr[:, off:off + P],
                                            tri_sb[:], ALU.mult,
                                        )
                                    nc.tensor.matmul(
                                        attn_ps[:, off:],
                                        v_sb[:, b * (T // P) + kt,
                                             h * HD:(h + 1) * HD],
                                        pr[:, off:],
                                        start=(kt == 0),
                                        stop=(kt == n_k - 1),
                                    )
                                    nc.tensor.matmul(
                                        sums_ps[:, off:],
                                        ones_sb[:],
                                        pr[:, off:],
                                        start=(kt == 0),
                                        stop=(kt == n_k - 1),
                                    )
                                # stash sums (DVE), evict attn unnormalized (DVE)
                                bidx = (b * HPC + h) * (T // QB) + j
                                po, fo = 32 * (bidx % 4), bidx // 4
                                nc.vector.tensor_copy(
                                    sums_sb[po:po + 1, fo, :], sums_ps[:]
                                )
                                nc.vector.tensor_copy(
                                    attn_sb[:, h, base + j * QB: base + (j + 1) * QB],
                                    attn_ps[:],
                                )

                            # ---- out-projection for this token block ----
                            tb = b * (T // QB) + j
                            for h in range(HPC):
                                bidx = (b * HPC + h) * (T // QB) + j
                                po, fo = 32 * (bidx % 4), bidx // 4
                                ln_row = nrmp.tile([1, QB], f32, tag="lnr",
                                                   name=f"lnr_{tb}_{h}")
                                nc.scalar.activation(
                                    ln_row[:], sums_sb[po:po + 1, fo, :], AF.Ln
                                )
                                rec_row = nrmp.tile([1, QB], f32, tag="rr",
                                                    name=f"rr_{tb}_{h}")
                                nc.scalar.activation(
                                    rec_row[:], ln_row[:], AF.Exp, scale=-1.0
                                )
                                rec_bc = nrmp.tile([P, QB], f32, tag="rbc",
                                                   name=f"rbc_{tb}_{h}")
                                nc.gpsimd.partition_broadcast(rec_bc[:], rec_row[:])
                                nc.vector.tensor_tensor(
                                    attn_sb[:, h, tb * QB:(tb + 1) * QB],
                                    attn_sb[:, h, tb * QB:(tb + 1) * QB],
                                    rec_bc[:], ALU.mult,
                                )
                            for m in range(H // P):
                                ps = ps3.tile([P, QB], f32, tag="out",
                                              name=f"out_{tb}_{m}")
                                for ko in range(HPC):
                                    nc.tensor.matmul(
                                        ps[:],
                                        wo_sb[:, ko, m * P:(m + 1) * P],
                                        attn_sb[:, ko, tb * QB:(tb + 1) * QB],
                                        start=(ko == 0),
                                        stop=(ko == HPC - 1),
                                    )
                                o_sb = o3p.tile([P, QB], f32, tag="osb",
                                                name=f"osb_{tb}_{m}")
                                if m % 2 == 0:
                                    nc.vector.tensor_copy(o_sb[:], ps[:])
                                else:
                                    nc.scalar.copy(o_sb[:], ps[:])
                                nc.sync.dma_start(
                                    outT[m * P:(m + 1) * P, tb * QB:(tb + 1) * QB],
                                    o_sb[:],
                                )
    nc.finalize()
    return nc


def _host_inputs(hidden_states, w_qkv, w_out):
    x = np.asarray(hidden_states, dtype=np.float32).reshape(TOK, H)
    w_qkv = np.asarray(w_qkv, dtype=np.float32)
    w_out = np.asarray(w_out, dtype=np.float32)

    xT = np.ascontiguousarray(x.T)  # [H, TOK]

    # lower-triangle-inclusive mask for the diagonal 128x128 strip
    tri = (np.arange(P)[:, None] <= np.arange(P)[None, :]).astype(np.float32)
    onesc = np.ones((P, 1), dtype=np.float32)

    in_maps = []
    for c in range(NCORES):
        heads = [HPC * c + i for i in range(HPC)]
        rows = []
        for sec in range(2):  # q, k sections
            for hh in heads:
                rows.append(w_qkv[sec * H + hh * HD: sec * H + (hh + 1) * HD])
        for hh in heads:  # v section
            rows.append(w_qkv[2 * H + hh * HD: 2 * H + (hh + 1) * HD])
        w_slice = np.concatenate(rows, axis=0)  # [768, H]
        wqkvT = np.ascontiguousarray(w_slice.T)  # [H, 768]
        cols = np.concatenate([np.arange(hh * HD, (hh + 1) * HD) for hh in heads])
        woutT = np.ascontiguousarray(w_out[:, cols].T)  # [256, H]
        in_maps.append({
            "xT": xT,
            "wqkvT": wqkvT,
            "woutT": woutT,
            "tri": tri,
            "onesc": onesc,
        })
    return in_maps


def _run(in_maps, trace=False):
    from concourse.bass_utils import run_bass_kernel_spmd

    if "nc" not in _CACHE:
        _CACHE["nc"] = _build()
    return run_bass_kernel_spmd(
        _CACHE["nc"], in_maps, core_ids=list(range(NCORES)), trace=trace
    )


def kernel(hidden_states, w_qkv, w_out):
    in_maps = _host_inputs(hidden_states, w_qkv, w_out)
    res = _run(in_maps)
    acc = res.results[0]["outT"].astype(np.float32)
    for c in range(1, NCORES):
        acc = acc + res.results[c]["outT"]
    out = np.ascontiguousarray(acc.T).reshape(B, T, H)
    return out.astype(np.float32)


# revision 24
# speedup vs baseline: 1.1251x; 1.1251x over previous
"""Trainium2 Bass kernel for dense multi-head causal self-attention.

Problem: hidden_states [2, 2048, 2048], w_qkv [6144, 2048], w_out [2048, 2048],
16 heads x 128 head_dim, causal softmax attention + out projection.

Sharding: tensor-parallel over heads. Each of the 8 cores computes 2 heads:
  - qkv projection for its 768 w_qkv rows (fp32r matmuls, x transposed on host);
    q,k produced transposed [d, tok], v produced natural [tok, d]
  - causal attention: scores transposed [k, q], exp on ScalarE over the
    causal-valid slice only, softmax denominators via a ones-column matmul on
    the PE, per-q-block normalize via gpsimd partition_broadcast
  - partial out-projection against its 256 w_out columns
Host sums the 8 partial outputs (the "all-reduce") and transposes back.
"""

import sys

sys.path.insert(0, "/opt/trn_rl_repo")

import numpy as np

B, T, H, NH, HD = 2, 2048, 2048, 16, 128
TOK = B * T  # 4096
P = 128
NCORES = 8
HPC = NH // NCORES  # heads per core = 2
SCALE = 1.0 / float(np.sqrt(HD))
QB = 512  # query block
KT = H // P  # 16 contraction tiles for qkv
NTB = TOK // QB  # 8 token blocks
NQK = 2 * HPC  # q,k output row-tiles per core
VW = HPC * HD  # v width (both heads) = 256

_CACHE = {}


def _build():
    import concourse.bacc as bacc
    import concourse.mybir as mybir
    import concourse.tile as tile

    dt = mybir.dt
    f32 = dt.float32
    f32r = dt.float32r
    AF = mybir.ActivationFunctionType
    ALU = mybir.AluOpType

    nc = bacc.Bacc(None, target_bir_lowering=False, debug=True)
    xT = nc.dram_tensor("xT", [H, TOK], f32r, kind="ExternalInput")
    wqkvT = nc.dram_tensor("wqkvT", [H, 6 * P], f32r, kind="ExternalInput")
    woutT = nc.dram_tensor("woutT", [HPC * P, H], f32r, kind="ExternalInput")
    tri = nc.dram_tensor("tri", [P, P], f32, kind="ExternalInput")
    onesc = nc.dram_tensor("onesc", [P, 1], f32r, kind="ExternalInput")
    outT = nc.dram_tensor("outT", [H, TOK], f32, kind="ExternalOutput")

    with tile.TileContext(nc) as tc:
        with tc.tile_pool(name="const", bufs=1) as constp, \
             tc.tile_pool(name="qk", bufs=1) as qkp:
            tri_sb = constp.tile([P, P], f32)
            nc.sync.dma_start(tri_sb[:], tri[:])
            ones_sb = constp.tile([P, 1], f32r)
            nc.sync.dma_start(ones_sb[:], onesc[:])

            qT_sb = qkp.tile([P, HPC, TOK], f32r)  # [d, h, tok]
            kT_sb = qkp.tile([P, HPC, TOK], f32r)  # [d, h, tok]
            v_sb = qkp.tile([P, TOK // P, VW], f32r)  # [tok%P, tok//P, h*HD+d]

            # ---------- Phase 1: qkv projection ----------
            # q,k transposed: psum[o,tok] = w_slice.T @ xT ; v natural:
            # psum[tok,d] = xT_chunk.T @ w_vT
            with tc.tile_pool(name="w1", bufs=1) as w1p, \
                 tc.tile_pool(name="x1", bufs=6) as x1p, \
                 tc.tile_pool(name="ps1", bufs=1, space="PSUM") as ps1, \
                 tc.tile_pool(name="psv", bufs=1, space="PSUM") as psv:
                w_sb = w1p.tile([P, KT, 6 * P], f32r)
                for tb in range(NTB):
                    ps_qk = [
                        ps1.tile([P, QB], f32, tag=f"psqk{m}", name=f"psqk{m}_{tb}")
                        for m in range(NQK)
                    ]
                    ps_v = [
                        psv.tile([P, VW], f32, tag=f"psv{c}", name=f"psv{c}_{tb}")
                        for c in range(4)
                    ]
                    for ko in range(KT):
                        if tb == 0:  # interleave weight loads with first block
                            nc.sync.dma_start(
                                w_sb[:, ko, :], wqkvT[ko * P:(ko + 1) * P, :]
                            )
                        x_t = x1p.tile([P, QB], f32r, tag="x")
                        nc.sync.dma_start(
                            x_t[:], xT[ko * P:(ko + 1) * P, tb * QB:(tb + 1) * QB]
                        )
                        for c in range(4):
                            nc.tensor.matmul(
                                ps_v[c][:],
                                x_t[:, c * P:(c + 1) * P],
                                w_sb[:, ko, NQK * P:],
                                start=(ko == 0),
                                stop=(ko == KT - 1),
                            )
                        for m in range(NQK):
                            nc.tensor.matmul(
                                ps_qk[m][:],
                                w_sb[:, ko, m * P:(m + 1) * P],
                                x_t[:],
                                start=(ko == 0),
                                stop=(ko == KT - 1),
                            )
                    for c in range(4):
                        nc.scalar.copy(v_sb[:, tb * 4 + c, :], ps_v[c][:])
                    for m in range(NQK):
                        if m < HPC:
                            dst = qT_sb[:, m, tb * QB:(tb + 1) * QB]
                        else:
                            dst = kT_sb[:, m - HPC, tb * QB:(tb + 1) * QB]
                        nc.vector.tensor_copy(dst, ps_qk[m][:])

            # ---------- Phases 2+3 (attn_sb reuses phase-1 pool space) ----------
            with tc.tile_pool(name="attn", bufs=1) as attnp, \
                 tc.tile_pool(name="w3", bufs=1) as w3p:
                attn_sb = attnp.tile([P, HPC, TOK], f32r)  # attn^T [d, h, tok]
                # one row per (b,h,j) block, packed 4 free-slots x 4
                # partition-offsets (engine APs need 32-aligned partitions)
                sums_sb = attnp.tile([P, 4, QB], f32)
                wo_sb = w3p.tile([P, HPC, H], f32r)
                for ko in range(HPC):
                    nc.sync.dma_start(
                        wo_sb[:, ko, :], woutT[ko * P:(ko + 1) * P, :]
                    )

                # Phases 2+3 fused: per (b, j) do both heads' attention,
                # then immediately normalize + out-project that token block.
                # Out-proj matmuls fill PE gaps while ScalarE runs exps; the
                # output DMA spreads across the whole timeline.
                with tc.tile_pool(name="pr", bufs=4) as prp, \
                     tc.tile_pool(name="sm", bufs=3) as smp, \
                     tc.tile_pool(name="o3", bufs=6) as o3p, \
                     tc.tile_pool(name="nrm", bufs=3) as nrmp, \
                     tc.tile_pool(name="ps_sc", bufs=3, space="PSUM") as ps_sc, \
                     tc.tile_pool(name="ps_at", bufs=2, space="PSUM") as ps_at, \
                     tc.tile_pool(name="ps_sm", bufs=1, space="PSUM") as ps_sm, \
                     tc.tile_pool(name="ps3", bufs=2, space="PSUM") as ps3:
                    for b in range(B):
                        base = b * T
                        for j in range(T // QB):
                            for h in range(HPC):
                                q_ap = qT_sb[:, h, base + j * QB: base + (j + 1) * QB]
                                n_k = (j + 1) * (QB // P)
                                attn_ps = ps_at.tile([P, QB], f32, tag="attn",
                                                     name=f"at_{b}_{h}_{j}")
                                sums_ps = ps_sm.tile([1, QB], f32, tag="sums",
                                                     name=f"sm_{b}_{h}_{j}")

                                def koff(kt, j=j):
                                    diag = kt - j * (QB // P)
                                    return diag * P if diag >= 0 else 0

                                def emit_scores(kt, b=b, h=h, j=j, q_ap=q_ap):
                                    off = koff(kt, j)
                                    sc = ps_sc.tile([P, QB], f32, tag="sc",
                                                    name=f"sc_{b}_{h}_{j}_{kt}")
                                    nc.tensor.matmul(
                                        sc[:, off:],
                                        kT_sb[:, h,
                                              base + kt * P: base + (kt + 1) * P],
                                        q_ap[:, off:],
                                        start=True,
                                        stop=True,
                                    )
                                    return sc

                                sc_q = [emit_scores(0)]
                                if n_k > 1:
                                    sc_q.append(emit_scores(1))
                                for kt in range(n_k):
                                    off = koff(kt)
                                    diag = kt - j * (QB // P)
                                    sc_cur = sc_q[kt]
                                    pr = prp.tile([P, QB], f32r, tag="pr",
                                                  name=f"pr_{b}_{h}_{j}_{kt}")
                                    nc.scalar.activation(
                                        pr[:, off:], sc_cur[:, off:],
                                        AF.Exp, scale=SCALE,
                                    )
                                    if kt + 2 < n_k:
                                        sc_q.append(emit_scores(kt + 2))
                                    if diag >= 0:  # mask the triangle strip
                                        nc.vector.tensor_tensor(
                                            pr[:, off:off + P], pr[:, off:off + P],
                                            tri_sb[:], ALU.mult,
                                        )
                                    nc.tensor.matmul(
                                        attn_ps[:, off:],
                                        v_sb[:, b * (T // P) + kt,
                                             h * HD:(h + 1) * HD],
                                        pr[:, off:],
                                        start=(kt == 0),
                                        stop=(kt == n_k - 1),
                                    )
                                    nc.tensor.matmul(
                                        sums_ps[:, off:],
                                        ones_sb[:],
                                        pr[:, off:],
                                        start=(kt == 0),
                                        stop=(kt == n_k - 1),
                                    )
                                # stash sums (DVE), evict attn unnormalized (DVE)
                                bidx = (b * HPC + h) * (T // QB) + j
                                po, fo = 32 * (bidx % 4), bidx // 4
                                nc.vector.tensor_copy(
                                    sums_sb[po:po + 1, fo, :], sums_ps[:]
                                )
                                nc.vector.tensor_copy(
                                    attn_sb[:, h, base + j * QB: base + (j + 1) * QB],
                                    attn_ps[:],
                                )

                            # ---- out-projection for this token block ----
                            tb = b * (T // QB) + j
                            for h in range(HPC):
                                bidx = (b * HPC + h) * (T // QB) + j
                                po, fo = 32 * (bidx % 4), bidx // 4
                                ln_row = nrmp.tile([1, QB], f32, tag="lnr",
                                                   name=f"lnr_{tb}_{h}")
                                nc.scalar.activation(
                                    ln_row[:], sums_sb[po:po + 1, fo, :], AF.Ln
                                )
                                rec_row = nrmp.tile([1, QB], f32, tag="rr",
                                                    name=f"rr_{tb}_{h}")
                                nc.scalar.activation(
                                    rec_row[:], ln_row[:], AF.Exp, scale=-1.0
                                )
                                rec_bc = nrmp.tile([P, QB], f32, tag="rbc",
                                                   name=f"rbc_{tb}_{h}")
                                nc.gpsimd.partition_broadcast(rec_bc[:], rec_row[:])
                                nc.vector.tensor_tensor(
                                    attn_sb[:, h, tb * QB:(tb + 1) * QB],
                                    attn_sb[:, h, tb * QB:(tb + 1) * QB],
                                    rec_bc[:], ALU.mult,
                                )
                            for m in range(H // P):
                                ps = ps3.tile([P, QB], f32, tag="out",
                                              name=f"out_{tb}_{m}")
                                for ko in range(HPC):
                                    nc.tensor.matmul(
                                        ps[:],
                                        wo_sb[:, ko, m * P:(m + 1) * P],
                                        attn_sb[:, ko, tb * QB:(tb + 1) * QB],
                                        start=(ko == 0),
                                        stop=(ko == HPC - 1),
                                    )
                                o_sb = o3p.tile([P, QB], f32, tag="osb",
                                                name=f"osb_{tb}_{m}")
                                if m % 2 == 0:
                                    nc.vector.tensor_copy(o_sb[:], ps[:])
                                else:
                                    nc.scalar.copy(o_sb[:], ps[:])
                                nc.sync.dma_start(
                                    outT[m * P:(m + 1) * P, tb * QB:(tb + 1) * QB],
                                    o_sb[:],
                                )
    nc.finalize()
    return nc


def _host_inputs(hidden_states, w_qkv, w_out):
    x = np.asarray(hidden_states, dtype=np.float32).reshape(TOK, H)
    w_qkv = np.asarray(w_qkv, dtype=np.float32)
    w_out = np.asarray(w_out, dtype=np.float32)

    xT = np.ascontiguousarray(x.T)  # [H, TOK]

    # lower-triangle-inclusive mask for the diagonal 128x128 strip
    tri = (np.arange(P)[:, None] <= np.arange(P)[None, :]).astype(np.float32)
    onesc = np.ones((P, 1), dtype=np.float32)

    in_maps = []
    for c in range(NCORES):
        heads = [HPC * c + i for i in range(HPC)]
        rows = []
        for sec in range(2):  # q, k sections
            for hh in heads:
                rows.append(w_qkv[sec * H + hh * HD: sec * H + (hh + 1) * HD])
        for hh in heads:  # v section
            rows.append(w_qkv[2 * H + hh * HD: 2 * H + (hh + 1) * HD])
        w_slice = np.concatenate(rows, axis=0)  # [768, H]
        wqkvT = np.ascontiguousarray(w_slice.T)  # [H, 768]
        cols = np.concatenate([np.arange(hh * HD, (hh + 1) * HD) for hh in heads])
        woutT = np.ascontiguousarray(w_out[:, cols].T)  # [256, H]
        in_maps.append({
            "xT": xT,
            "wqkvT": wqkvT,
            "woutT": woutT,
            "tri": tri,
            "onesc": onesc,
        })
    return in_maps


def _run(in_maps, trace=False):
    from concourse.bass_utils import run_bass_kernel_spmd

    if "nc" not in _CACHE:
        _CACHE["nc"] = _build()
    return run_bass_kernel_spmd(
        _CACHE["nc"], in_maps, core_ids=list(range(NCORES)), trace=trace
    )


def kernel(hidden_states, w_qkv, w_out):
    in_maps = _host_inputs(hidden_states, w_qkv, w_out)
    res = _run(in_maps)
    acc = res.results[0]["outT"].astype(np.float32)
    for c in range(1, NCORES):
        acc = acc + res.results[c]["outT"]
    out = np.ascontiguousarray(acc.T).reshape(B, T, H)
    return out.astype(np.float32)
